# revision 11
# baseline (speedup 1.0000x reference)
"""Bass/Trainium2 kernel for nn_Attention_13615046328582.

Causal multi-head attention with RoPE, B=4 S=2048 E=2048 H=16 D=128, fp32 io.
Sharding: 4-way batch DP x 2-way head TP across 8 NeuronCores.
Each core: 1 batch, 8 heads. Host sums the TP pair partials + bo.

All matmuls bf16 (same PE rate as fp32r; FWL hides weight loads, DMA/SBUF
halve). Numerics sim-validated: metric ~4e-3 vs the 2e-2 gate.

Per-core plan:
  P1  V/Q/K projections; x streamed per 512-t block (2 blocks live), weights
      in 2MB halves through a 3-slot pool. Stationaries serve 2 moving
      blocks each to amortize LDWEIGHTS. Q/K features host-permuted to
      [evens|odds] per head so RoPE's pair swap is a half-partition swap:
      SBUF->SBUF DMA on the gpsimd queue + 3 DVE ops. Q/K/V all land
      resident in SBUF (no DRAM spill).
  P2+P3 fused, blocks outer / heads inner. Per (block, head): fine-grained
      causal logits (diagonal straddle tiles stream only their valid
      q-suffix; true diagonal masked by bf16 triangle mul), exp on ACT
      batched in 2-bank pairs, row sums via ones-vector matmuls, 1/sums via
      DVE reciprocal_approx_fast + gpsimd partition_broadcast, normalize
      deferred one head so PE never waits. After each block's heads, the
      out-projection for its 4 t-tiles runs immediately from SBUF (wo
      prefetched during block 0), filling ACT-bound bubbles; f32 out.
"""

import numpy as np
import ml_dtypes
from contextlib import ExitStack

import concourse.bass as bass
import concourse.tile as tile
from concourse import bacc, mybir
from concourse.bass_utils import run_bass_kernel_spmd

B, S, E, H = 4, 2048, 2048, 16
D = E // H            # 128 head dim
HL = 8                # heads per core
FL = HL * D           # 1024 local features
N_CORES = 8
ROPE_BASE = 10000.0
SCALE = float(D) ** -0.5
F32 = mybir.dt.float32
F32R = mybir.dt.float32r
BF16 = mybir.dt.bfloat16
Act = mybir.ActivationFunctionType

NE = E // 128         # 16 e-tiles
NT = S // 512         # 4 t-blocks of 512
NTT = S // 128        # 16 t-tiles of 128
NFO = FL // 128       # 8 f-tiles (= local heads)


def build_nc():
    nc = bacc.Bacc(
        "TRN2", target_bir_lowering=False, debug=False, num_devices=N_CORES
    )
    xT = nc.dram_tensor("xT", [E, S], BF16, kind="ExternalInput").ap()
    wq = nc.dram_tensor("wq", [E, FL], BF16, kind="ExternalInput").ap()
    wk = nc.dram_tensor("wk", [E, FL], BF16, kind="ExternalInput").ap()
    wv = nc.dram_tensor("wv", [E, FL], BF16, kind="ExternalInput").ap()
    wo = nc.dram_tensor("wo", [FL, E], BF16, kind="ExternalInput").ap()
    bq = nc.dram_tensor("bq", [128, NFO], F32, kind="ExternalInput").ap()
    bk = nc.dram_tensor("bk", [128, NFO], F32, kind="ExternalInput").ap()
    bv = nc.dram_tensor("bv", [1, FL], BF16, kind="ExternalInput").ap()
    cosT = nc.dram_tensor("cosT", [128, S], BF16, kind="ExternalInput").ap()
    sinST = nc.dram_tensor("sinST", [128, S], BF16, kind="ExternalInput").ap()
    tri = nc.dram_tensor("tri", [128, 128], BF16, kind="ExternalInput").ap()
    ones_col_d = nc.dram_tensor("ones_col", [128, 1], BF16, kind="ExternalInput").ap()
    out = nc.dram_tensor("out", [S, E], F32, kind="ExternalOutput").ap()

    with tile.TileContext(nc) as tc, ExitStack() as top:
        cpool = top.enter_context(tc.tile_pool(name="const", bufs=1))
        ones_col = cpool.tile([128, 1], BF16, tag="ones_col")
        nc.sync.dma_start(ones_col[:], ones_col_d[:])
        zb = cpool.tile([128, 1], F32, tag="zb")
        nc.gpsimd.memset(zb[:], 0.0)
        tri_sb = cpool.tile([128, 128], BF16, tag="tri")
        nc.sync.dma_start(tri_sb[:], tri[:])
        bq_sb = cpool.tile([128, NFO], F32, tag="bq")
        nc.sync.dma_start(bq_sb[:], bq[:])
        bk_sb = cpool.tile([128, NFO], F32, tag="bk")
        nc.sync.dma_start(bk_sb[:], bk[:])
        bv_sb = cpool.tile([1, FL], BF16, tag="bv")
        nc.sync.dma_start(bv_sb[:], bv[:])

        # Whole-kernel residents: rope'd Q/K and V (all consumed by P2).
        res = top.enter_context(tc.tile_pool(name="res", bufs=1))
        qT_res = res.tile([128, NFO, S], BF16, tag="qT")
        kT_res = res.tile([128, NFO, S], BF16, tag="kT")
        v_res = res.tile([128, NTT, FL], BF16, tag="v")

        xT_r = xT.rearrange("(eo p) t -> p eo t", p=128)

        # ---- Phase 1: V/Q/K projections (V first: it plus head 0 of Q/K
        # gates P2's start).
        with ExitStack() as ph:
            c1 = ph.enter_context(tc.tile_pool(name="c1", bufs=1))
            wp = ph.enter_context(tc.tile_pool(name="wqk", bufs=3))
            xp = ph.enter_context(tc.tile_pool(name="xs", bufs=2))
            ps = ph.enter_context(tc.tile_pool(name="ps1", bufs=4, space="PSUM"))
            st = ph.enter_context(tc.tile_pool(name="st1", bufs=3))

            def load_w_half(w_in, half):
                t = wp.tile([128, NE, 512], BF16, tag="w")
                nc.sync.dma_start(
                    t[:],
                    w_in.rearrange("(eo p) f -> p eo f", p=128)[
                        :, :, half * 512 : (half + 1) * 512
                    ],
                )
                return t

            def load_x(tb):
                t = xp.tile([128, NE, 512], BF16, tag="x")
                nc.sync.dma_start(t[:], xT_r[:, :, tb * 512 : (tb + 1) * 512])
                return t

            # V pass: stationary x-block serves both f-halves (2 MMs/LDW).
            wv_half = [load_w_half(wv, 0)]
            x_sb = load_x(0)
            wv_half.append(load_w_half(wv, 1))
            for tb in range(NT):
                if tb > 0:
                    x_sb = load_x(tb)
                for ttl in range(4):
                    tt = tb * 4 + ttl
                    accs = [ps.tile([128, 512], F32, name=f"acc{i}") for i in range(2)]
                    for eo in range(NE):
                        for fo2 in range(2):
                            nc.tensor.matmul(
                                accs[fo2][:],
                                x_sb[:, eo, ttl * 128 : (ttl + 1) * 128],
                                wv_half[fo2][:, eo, :],
                                start=(eo == 0),
                                stop=(eo == NE - 1),
                            )
                    for fo2 in range(2):
                        nc.scalar.copy(
                            v_res[:, tt, fo2 * 512 : (fo2 + 1) * 512],
                            accs[fo2][:],
                        )

            cos_sb = c1.tile([128, S], BF16, tag="cos")
            nc.sync.dma_start(cos_sb[:], cosT[:])
            sin_sb = c1.tile([128, S], BF16, tag="sin")
            nc.sync.dma_start(sin_sb[:], sinST[:])

            # Q and K passes: stationary w(fo,eo) serves 2 t-blocks
            # (2 MMs/LDW); bias via ACT evac; RoPE on DVE with the half-
            # partition swap done by SBUF->SBUF DMA on the gpsimd queue.
            for w_in, b_sb, dst in ((wq, bq_sb, qT_res), (wk, bk_sb, kT_res)):
                w_half = [load_w_half(w_in, 0), load_w_half(w_in, 1)]
                for tbp in range(2):
                    x2 = [load_x(2 * tbp), load_x(2 * tbp + 1)]
                    for fo in range(NFO):
                        w_sb = w_half[fo // 4]
                        fl = (fo % 4) * 128
                        accs = [ps.tile([128, 512], F32, name=f"acc{i}") for i in range(2)]
                        for eo in range(NE):
                            for i in range(2):
                                nc.tensor.matmul(
                                    accs[i][:],
                                    w_sb[:, eo, fl : fl + 128],
                                    x2[i][:, eo, :],
                                    start=(eo == 0),
                                    stop=(eo == NE - 1),
                                )
                        for i in range(2):
                            tb = 2 * tbp + i
                            ts = slice(tb * 512, (tb + 1) * 512)
                            raw = st.tile([128, 512], BF16, tag="raw")
                            nc.scalar.activation(
                                raw[:], accs[i][:], Act.Identity,
                                bias=b_sb[:, fo : fo + 1],
                            )
                            raws = st.tile([128, 512], BF16, tag="raws")
                            nc.gpsimd.dma_start(raws[0:64, :], raw[64:128, :])
                            nc.gpsimd.dma_start(raws[64:128, :], raw[0:64, :])
                            u = st.tile([128, 512], BF16, tag="u")
                            nc.vector.tensor_mul(u[:], raw[:], cos_sb[:, ts])
                            w2 = st.tile([128, 512], BF16, tag="w2")
                            nc.vector.tensor_mul(w2[:], raws[:], sin_sb[:, ts])
                            nc.vector.tensor_add(dst[:, fo, ts], u[:], w2[:])

        # ---- Phase 2+3 fused: attention blocks + immediate out-projection.
        wp3 = top.enter_context(tc.tile_pool(name="wo", bufs=1))
        wo_sb = wp3.tile([128, NFO, E], BF16)
        wo_r = wo.rearrange("(fo p) e -> p fo e", p=128)
        for fo in range(NFO):
            nc.sync.dma_start(wo_sb[:, fo, :], wo_r[:, fo, :])

        with ExitStack() as ph:
            aop = ph.enter_context(tc.tile_pool(name="aob", bufs=2))
            ep = ph.enter_context(tc.tile_pool(name="expS", bufs=1))
            psS = ph.enter_context(tc.tile_pool(name="psS", bufs=2, space="PSUM"))
            psSum = ph.enter_context(
                tc.tile_pool(name="psSum", bufs=1, space="PSUM")
            )
            psO = ph.enter_context(tc.tile_pool(name="psO", bufs=2, space="PSUM"))
            ps3 = ph.enter_context(tc.tile_pool(name="ps3", bufs=1, space="PSUM"))
            sm = ph.enter_context(tc.tile_pool(name="sm2", bufs=2))
            st = ph.enter_context(tc.tile_pool(name="st2", bufs=3))

            pend = None  # deferred normalize: (pso, recip, ao_blk, h)

            def flush_normalize():
                nonlocal pend
                if pend is None:
                    return
                pso, recip, ao_blk, h = pend
                bcast = sm.tile([128, 512], F32, tag="bcast")
                nc.gpsimd.partition_broadcast(bcast[:], recip[:])
                nc.vector.tensor_mul(ao_blk[:, h, :], pso[:], bcast[:])
                pend = None

            for b in range(NT):
                nk = 4 * b + 4
                ao_blk = aop.tile([128, NFO, 512], BF16)
                for h in range(HL):
                    eS = ep.tile([128, NTT, 512], BF16)
                    # logits + exp (full tiles paired across 2 PSUM banks)
                    j = 0
                    while j < nk:
                        m = j - 4 * b
                        if m < 0 and j + 1 < 4 * b:
                            ls = psS.tile([128, 2, 512], F32)
                            for jj in range(2):
                                nc.tensor.matmul(
                                    ls[:, jj, :],
                                    kT_res[:, h, (j + jj) * 128 : (j + jj + 1) * 128],
                                    qT_res[:, h, b * 512 : (b + 1) * 512],
                                    start=True, stop=True,
                                )
                            nc.scalar.activation(
                                eS[:, j : j + 2, :], ls[:, :, :], Act.Exp,
                                bias=zb[:, 0:1], scale=SCALE,
                            )
                            j += 2
                        else:
                            q0 = 128 * max(m, 0)
                            ls = psS.tile([128, 2, 512], F32)
                            nc.tensor.matmul(
                                ls[:, 0, q0:512],
                                kT_res[:, h, j * 128 : (j + 1) * 128],
                                qT_res[:, h, b * 512 + q0 : (b + 1) * 512],
                                start=True, stop=True,
                            )
                            nc.scalar.activation(
                                eS[:, j, q0:512], ls[:, 0, q0:512], Act.Exp,
                                bias=zb[:, 0:1], scale=SCALE,
                            )
                            if m >= 0:  # true-diagonal 128x128 triangle
                                nc.vector.tensor_mul(
                                    eS[:, j, q0 : q0 + 128],
                                    eS[:, j, q0 : q0 + 128],
                                    tri_sb[:],
                                )
                            j += 1
                    # row sums
                    ssum = psSum.tile([1, 512], F32)
                    for j in range(nk):
                        q0 = 128 * max(j - 4 * b, 0)
                        nc.tensor.matmul(
                            ssum[:, q0:512], ones_col[:], eS[:, j, q0:512],
                            start=(j == 0), stop=(j == nk - 1),
                        )
                    sums_bf = sm.tile([1, 512], BF16, tag="sums")
                    nc.scalar.copy(sums_bf[:], ssum[:])
                    recip = sm.tile([1, 512], F32, tag="recip")
                    nc.vector.reciprocal_approx_fast(recip[:], ssum[:])
                    # PV (+ rank-1 bv*sums fold)
                    pso = psO.tile([128, 512], F32)
                    for j in range(nk):
                        q0 = 128 * max(j - 4 * b, 0)
                        nc.tensor.matmul(
                            pso[:, q0:512],
                            v_res[:, j, h * 128 : (h + 1) * 128],
                            eS[:, j, q0:512],
                            start=(j == 0), stop=False,
                        )
                    nc.tensor.matmul(
                        pso[:],
                        bv_sb[0:1, h * 128 : (h + 1) * 128],
                        sums_bf[:],
                        start=False, stop=True,
                    )
                    flush_normalize()
                    pend = (pso, recip, ao_blk, h)
                flush_normalize()
                # out-projection for this block's 4 t-tiles
                for tl in range(4):
                    tt = 4 * b + tl
                    for eb in range(NT):
                        acc = ps3.tile([128, 512], F32)
                        for fo in range(NFO):
                            nc.tensor.matmul(
                                acc[:],
                                ao_blk[:, fo, tl * 128 : (tl + 1) * 128],
                                wo_sb[:, fo, eb * 512 : (eb + 1) * 512],
                                start=(fo == 0),
                                stop=(fo == NFO - 1),
                            )
                        osb = st.tile([128, 512], F32, tag="osb")
                        nc.vector.tensor_copy(osb[:], acc[:])
                        nc.sync.dma_start(
                            out[tt * 128 : (tt + 1) * 128,
                                eb * 512 : (eb + 1) * 512],
                            osb[:],
                        )

    nc.compile()
    return nc


def _host_inputs(x, Wq, bq, Wk, bk, Wv, bv, Wo, bo):
    BF = ml_dtypes.bfloat16
    # Per-head feature permutation: evens then odds (RoPE half-swap layout).
    perm1 = np.concatenate([np.arange(0, D, 2), np.arange(1, D, 2)])
    i = np.arange(0, D, 2, dtype=np.float64)
    invf = ROPE_BASE ** (-i / D)                      # (64,)
    pos = np.arange(S, dtype=np.float64)
    ang = pos[None, :] * invf[:, None]                # (64, S)
    cosT = np.concatenate([np.cos(ang), np.cos(ang)], 0).astype(np.float32)
    # sin table signs match the half-SWAPPED operand (raws): row i<64 holds
    # -sin_i (multiplies o_i), row 64+i holds +sin_i (multiplies e_i).
    sinST = np.concatenate([-np.sin(ang), np.sin(ang)], 0).astype(np.float32)
    ki = np.arange(128)[:, None]
    qi = np.arange(128)[None, :]
    tri = (qi >= ki).astype(np.float32)

    in_maps = []
    for c in range(N_CORES):
        bi, g = c % B, c // B
        sl = slice(g * FL, (g + 1) * FL)
        fperm = (np.arange(FL).reshape(NFO, D)[:, perm1]).reshape(FL)
        wq_l = Wq[sl, :][fperm, :]
        wk_l = Wk[sl, :][fperm, :]
        bq_l = bq[sl][fperm]
        bk_l = bk[sl][fperm]
        in_maps.append({
            "xT": np.ascontiguousarray(x[bi].T).astype(BF),
            "wq": np.ascontiguousarray(wq_l.T).astype(BF),
            "wk": np.ascontiguousarray(wk_l.T).astype(BF),
            "wv": np.ascontiguousarray(Wv[sl, :].T).astype(BF),
            "wo": np.ascontiguousarray(Wo[:, sl].T).astype(BF),
            "bq": np.ascontiguousarray(bq_l.reshape(NFO, 128).T).astype(np.float32),
            "bk": np.ascontiguousarray(bk_l.reshape(NFO, 128).T).astype(np.float32),
            "bv": bv[sl].reshape(1, FL).astype(BF),
            "cosT": cosT.astype(BF),
            "sinST": sinST.astype(BF),
            "tri": tri.astype(BF),
            "ones_col": np.ones((128, 1), np.float32).astype(BF),
        })
    return in_maps


_NC_CACHE = {}


def _install_ntff_hook():
    """Recreate the missing antenv.axon_hooks module so trace=True works."""
    import sys, types, ctypes, contextlib

    if "antenv.axon_hooks" in sys.modules:
        return
    so_path = "/opt/axon/libaxon_pjrt.so"
    lib = ctypes.CDLL(so_path)
    lib.axon_start_nrt_profile.argtypes = [
        ctypes.POINTER(ctypes.c_int64), ctypes.c_size_t,
    ]
    lib.axon_start_nrt_profile.restype = ctypes.c_int64
    lib.axon_stop_nrt_profile.argtypes = [ctypes.c_char_p]
    lib.axon_stop_nrt_profile.restype = ctypes.c_int64

    @contextlib.contextmanager
    def _hook(output_dir, device_ids):
        import jax
        jax.devices()
        if device_ids:
            ids = (ctypes.c_int64 * len(device_ids))(*device_ids)
            rc = lib.axon_start_nrt_profile(ids, len(device_ids))
        else:
            rc = lib.axon_start_nrt_profile(None, 0)
        if rc != 0:
            raise RuntimeError(f"axon_start_nrt_profile rc={rc}")
        try:
            yield
        finally:
            n = lib.axon_stop_nrt_profile(str(output_dir).encode())
            print(f"ntff profile: {n} file(s) -> {output_dir}", flush=True)

    mod = types.ModuleType("antenv.axon_hooks")
    mod.get_axon_ntff_profile_hook = lambda: _hook
    mod.set_axon_ntff_profile_hook = lambda h: None
    sys.modules["antenv.axon_hooks"] = mod
    import concourse.bass_utils as bu
    bu.upload_artifacts = lambda tmpdir: f"file://{tmpdir}"


def run(x, Wq, bq, Wk, bk, Wv, bv, Wo, bo, trace=False):
    if trace:
        try:
            _install_ntff_hook()
        except Exception as e:
            print(f"ntff hook install failed ({e}); tracing may degrade")
    if "nc" not in _NC_CACHE:
        _NC_CACHE["nc"] = build_nc()
    nc = _NC_CACHE["nc"]
    in_maps = _host_inputs(x, Wq, bq, Wk, bk, Wv, bv, Wo, bo)
    res = run_bass_kernel_spmd(
        nc, in_maps, core_ids=list(range(N_CORES)), trace=trace
    )
    outs = [res.results[c]["out"] for c in range(N_CORES)]
    full = np.empty((B, S, E), np.float32)
    for bi in range(B):
        full[bi] = outs[bi] + outs[bi + B] + bo[None, :]
    return full, res


def kernel(**inputs):
    full, _ = run(**inputs)
    return full


# revision 27
# speedup vs baseline: 1.0978x; 1.0978x over previous
"""Bass/Trainium2 kernel for nn_Attention_13615046328582.

Causal multi-head attention with RoPE, B=4 S=2048 E=2048 H=16 D=128, fp32 io.
Sharding: 4-way batch DP x 2-way head TP across 8 NeuronCores.
Each core: 1 batch, 8 heads. Host sums the TP pair partials + bo.

All matmuls bf16 (same PE rate as fp32r; FWL hides weight loads, DMA/SBUF
halve). fp8/DoubleRow was evaluated and rejected: numpy simulation of e4m3
on any single GEMM stage gives 2.7-4.6e-2 max-err metric, over the 2e-2
gate. bf16-everything sims at 4.2e-3 and measures 4.5e-3 on HW.

Measured (NTFF, warm 2.4GHz clock): 734us/core NEFF wall, PE ~93% busy
(vs 950us for the fp32r baseline). PE streaming floor is ~625us; the rest
is per-MM issue latency, ~17us cold-start DMA, and ~12us teardown fences.
Beware: the shared device occasionally drops to 2.0GHz (P0 power state),
inflating runs by ~20% - check inter-MM gap (216ns = warm 2.4GHz).

Per-core plan:
  P1  V/Q/K projections; x streamed per 512-t block on the vector DMA queue
      (weights on sync/scalar queues so the cold-start loads run in
      parallel). Q/K features host-permuted to [evens|odds] per head so
      RoPE's pair swap is a half-partition swap: SBUF->SBUF DMA on the
      gpsimd queue + 3 DVE ops. Q/K land in per-head resident tiles (so P2
      head 0 doesn't wait on the whole pass); V resident too. No DRAM
      spill.
  P2+P3 fused, blocks outer / heads inner. Per (block, head): fine-grained
      causal logits (diagonal-straddle tiles stream only their valid
      q-suffix; true diagonal masked by a bf16 triangle mul), exp on ACT
      batched in 2-bank pairs, row sums via ones-vector matmuls, 1/sums via
      DVE reciprocal_approx_fast, broadcast via gpsimd partition_broadcast
      (no PE involvement), normalize on DVE straight into the block's ao
      tile. After each block's heads, the out-projection for its 4 t-tiles
      runs from SBUF (wo prefetched during block 0); f32 out.
"""

import numpy as np
import ml_dtypes
from contextlib import ExitStack

import concourse.bass as bass
import concourse.tile as tile
from concourse import bacc, mybir
from concourse.bass_utils import run_bass_kernel_spmd

B, S, E, H = 4, 2048, 2048, 16
D = E // H            # 128 head dim
HL = 8                # heads per core
FL = HL * D           # 1024 local features
N_CORES = 8
ROPE_BASE = 10000.0
SCALE = float(D) ** -0.5
F32 = mybir.dt.float32
F32R = mybir.dt.float32r
BF16 = mybir.dt.bfloat16
Act = mybir.ActivationFunctionType

NE = E // 128         # 16 e-tiles
NT = S // 512         # 4 t-blocks of 512
NTT = S // 128        # 16 t-tiles of 128
NFO = FL // 128       # 8 f-tiles (= local heads)


def build_nc():
    nc = bacc.Bacc(
        "TRN2", target_bir_lowering=False, debug=False, num_devices=N_CORES
    )
    # x and projection weights arrive host-pre-tiled so every chunk DMA is
    # a fully contiguous [128 x 8KB] transfer:
    #   xT[p, tb*NE*512 + eo*512 + tl] = x[tb*512+tl, eo*128+p]
    #   w*[p, half*NE*512 + eo*512 + f] = W.T[eo*128+p, half*512+f]
    xT = nc.dram_tensor("xT", [128, NT * NE * 512], BF16, kind="ExternalInput").ap()
    wq = nc.dram_tensor("wq", [128, 2 * NE * 512], BF16, kind="ExternalInput").ap()
    wk = nc.dram_tensor("wk", [128, 2 * NE * 512], BF16, kind="ExternalInput").ap()
    wv = nc.dram_tensor("wv", [128, 2 * NE * 512], BF16, kind="ExternalInput").ap()
    wo = nc.dram_tensor("wo", [FL, E], BF16, kind="ExternalInput").ap()
    bq = nc.dram_tensor("bq", [128, NFO], F32, kind="ExternalInput").ap()
    bk = nc.dram_tensor("bk", [128, NFO], F32, kind="ExternalInput").ap()
    bv = nc.dram_tensor("bv", [1, FL], BF16, kind="ExternalInput").ap()
    cosT = nc.dram_tensor("cosT", [128, S], BF16, kind="ExternalInput").ap()
    sinST = nc.dram_tensor("sinST", [128, S], BF16, kind="ExternalInput").ap()
    tri = nc.dram_tensor("tri", [128, 128], BF16, kind="ExternalInput").ap()
    ones_col_d = nc.dram_tensor("ones_col", [128, 1], BF16, kind="ExternalInput").ap()
    out = nc.dram_tensor("out", [S, E], F32, kind="ExternalOutput").ap()

    with tile.TileContext(nc) as tc, ExitStack() as top:
        cpool = top.enter_context(tc.tile_pool(name="const", bufs=1))
        ones_col = cpool.tile([128, 1], BF16, tag="ones_col")
        nc.sync.dma_start(ones_col[:], ones_col_d[:])
        zb = cpool.tile([128, 1], F32, tag="zb")
        nc.gpsimd.memset(zb[:], 0.0)
        # Warm the gpsimd Q7 partition_broadcast library now (first use pays
        # a ~12us program load; do it while PE waits on the initial DMAs).
        warm_in = cpool.tile([1, 8], F32, tag="warm_in")
        nc.gpsimd.memset(warm_in[:], 1.0)
        warm_out = cpool.tile([128, 8], F32, tag="warm_out")
        nc.gpsimd.partition_broadcast(warm_out[:], warm_in[:])
        tri_sb = cpool.tile([128, 128], BF16, tag="tri")
        nc.sync.dma_start(tri_sb[:], tri[:])
        bq_sb = cpool.tile([128, NFO], F32, tag="bq")
        nc.sync.dma_start(bq_sb[:], bq[:])
        bk_sb = cpool.tile([128, NFO], F32, tag="bk")
        nc.sync.dma_start(bk_sb[:], bk[:])
        bv_sb = cpool.tile([1, FL], BF16, tag="bv")
        nc.sync.dma_start(bv_sb[:], bv[:])

        # Whole-kernel residents. Q/K per head so P2's head h only waits on
        # head h's writes (tile-granular deps); V one tile (fully written
        # long before first use).
        res = top.enter_context(tc.tile_pool(name="res", bufs=1))
        qT_res = [res.tile([128, S], BF16, tag=f"qT{h}", name=f"qT{h}")
                  for h in range(NFO)]
        kT_res = [res.tile([128, S], BF16, tag=f"kT{h}", name=f"kT{h}")
                  for h in range(NFO)]
        v_res = res.tile([128, NTT, FL], BF16, tag="v")



        # ---- Phase 1: V/Q/K projections (V first: it gates nothing until
        # P2, but its weights are the cold-start load).
        with ExitStack() as ph:
            c1 = ph.enter_context(tc.tile_pool(name="c1", bufs=1))
            wp = ph.enter_context(tc.tile_pool(name="wqk", bufs=6))
            xp = ph.enter_context(tc.tile_pool(name="xs", bufs=4))
            ps = ph.enter_context(tc.tile_pool(name="ps1", bufs=6, space="PSUM"))
            st = ph.enter_context(tc.tile_pool(name="st1", bufs=3))

            # x and w stream as [128, 8, 512] eo-half chunks (separate tiles
            # so the first matmuls start as soon as their chunk lands, not
            # after the whole 2MB half); each chunk is one contiguous
            # [128 x 8KB] DMA thanks to the host pre-tiling.
            def load_w_half(w_in, half, eng=None):
                ts = []
                for c in range(2):
                    t = wp.tile([128, 8, 512], BF16, tag="w", name=f"w{c}")
                    off = (half * NE + c * 8) * 512
                    (eng or nc.sync).dma_start(
                        t[:], w_in[:, off : off + 8 * 512]
                    )
                    ts.append(t)
                return ts

            def load_x(tb):
                ts = []
                for c in range(2):
                    t = xp.tile([128, 8, 512], BF16, tag="x", name=f"x{c}")
                    off = (tb * NE + c * 8) * 512
                    # scalar queue: parallel with the weight loads on sync
                    nc.scalar.dma_start(t[:], xT[:, off : off + 8 * 512])
                    ts.append(t)
                return ts

            # Cold start is HBM-aggregate bound: land the critical 4MB
            # (wv half 0 + x block 0) first; wv half 1 queues behind wv0 on
            # sync and streams in during the fo2=0 tiles.
            wv_half = [load_w_half(wv, 0)]
            x_sb = load_x(0)
            wv_half.append(load_w_half(wv, 1))
            for tb in range(NT):
                if tb > 0:
                    x_sb = load_x(tb)
                # tb=0 runs all fo2=0 tiles first: the first ~14us of MMs
                # only need wv half 0, halving the cold-start DMA pressure.
                order = ([(ttl, 0) for ttl in range(4)]
                         + [(ttl, 1) for ttl in range(4)]) if tb == 0 else \
                        [(ttl, fo2) for ttl in range(4) for fo2 in range(2)]
                for ttl, fo2 in order:
                    tt = tb * 4 + ttl
                    acc = ps.tile([128, 512], F32)
                    for eo in range(NE):
                        nc.tensor.matmul(
                            acc[:],
                            x_sb[eo // 8][:, eo % 8,
                                          ttl * 128 : (ttl + 1) * 128],
                            wv_half[fo2][eo // 8][:, eo % 8, :],
                            start=(eo == 0),
                            stop=(eo == NE - 1),
                        )
                    nc.scalar.copy(
                        v_res[:, tt, fo2 * 512 : (fo2 + 1) * 512], acc[:]
                    )

            cos_sb = c1.tile([128, S], BF16, tag="cos")
            nc.sync.dma_start(cos_sb[:], cosT[:])
            sin_sb = c1.tile([128, S], BF16, tag="sin")
            nc.sync.dma_start(sin_sb[:], sinST[:])

            # Q and K passes: bias via ACT evac; RoPE on DVE with the half-
            # partition swap done by SBUF->SBUF DMA on the gpsimd queue.
            for w_in, b_sb, dst in ((wq, bq_sb, qT_res), (wk, bk_sb, kT_res)):
                w_half = [load_w_half(w_in, 0), load_w_half(w_in, 1)]
                for tb in range(NT):
                    x_sb = load_x(tb)
                    ts = slice(tb * 512, (tb + 1) * 512)
                    for fo in range(NFO):
                        w_sb = w_half[fo // 4]
                        fl = (fo % 4) * 128
                        acc = ps.tile([128, 512], F32)
                        for eo in range(NE):
                            nc.tensor.matmul(
                                acc[:],
                                w_sb[eo // 8][:, eo % 8, fl : fl + 128],
                                x_sb[eo // 8][:, eo % 8, :],
                                start=(eo == 0),
                                stop=(eo == NE - 1),
                            )
                        raw = st.tile([128, 512], BF16, tag="raw")
                        nc.scalar.activation(
                            raw[:], acc[:], Act.Identity,
                            bias=b_sb[:, fo : fo + 1],
                        )
                        raws = st.tile([128, 512], BF16, tag="raws")
                        nc.gpsimd.dma_start(raws[0:64, :], raw[64:128, :])
                        nc.gpsimd.dma_start(raws[64:128, :], raw[0:64, :])
                        u = st.tile([128, 512], BF16, tag="u")
                        nc.vector.tensor_mul(u[:], raw[:], cos_sb[:, ts])
                        w2 = st.tile([128, 512], BF16, tag="w2")
                        nc.vector.tensor_mul(w2[:], raws[:], sin_sb[:, ts])
                        nc.vector.tensor_add(dst[fo][:, ts], u[:], w2[:])

        # ---- Phase 2+3 fused: attention blocks + immediate out-projection.
        wp3 = top.enter_context(tc.tile_pool(name="wo", bufs=1))
        wo_sb = wp3.tile([128, NFO, E], BF16)
        wo_r = wo.rearrange("(fo p) e -> p fo e", p=128)
        for fo in range(NFO):
            nc.sync.dma_start(wo_sb[:, fo, :], wo_r[:, fo, :])

        with ExitStack() as ph:
            aop = ph.enter_context(tc.tile_pool(name="aob", bufs=2))
            ep = ph.enter_context(tc.tile_pool(name="expS", bufs=2))
            psS = ph.enter_context(tc.tile_pool(name="psS", bufs=3, space="PSUM"))
            psSum = ph.enter_context(
                tc.tile_pool(name="psSum", bufs=1, space="PSUM")
            )
            psO = ph.enter_context(tc.tile_pool(name="psO", bufs=2, space="PSUM"))
            ps3 = ph.enter_context(tc.tile_pool(name="ps3", bufs=2, space="PSUM"))
            sm = ph.enter_context(tc.tile_pool(name="sm2", bufs=2))
            st = ph.enter_context(tc.tile_pool(name="st2", bufs=3))

            for b in range(NT):
                nk = 4 * b + 4
                ao_blk = aop.tile([128, NFO, 512], BF16)
                for h in range(HL):
                    eS = ep.tile([128, NTT, 512], BF16)
                    # logits + exp
                    for j in range(nk):
                        m = j - 4 * b
                        q0 = 128 * max(m, 0)
                        ls = psS.tile([128, 512], F32)
                        nc.tensor.matmul(
                            ls[:, q0:512],
                            kT_res[h][:, j * 128 : (j + 1) * 128],
                            qT_res[h][:, b * 512 + q0 : (b + 1) * 512],
                            start=True, stop=True,
                        )
                        nc.scalar.activation(
                            eS[:, j, q0:512], ls[:, q0:512], Act.Exp,
                            bias=zb[:, 0:1], scale=SCALE,
                        )
                        if m >= 0:  # true-diagonal 128x128 triangle
                            nc.vector.tensor_mul(
                                eS[:, j, q0 : q0 + 128],
                                eS[:, j, q0 : q0 + 128],
                                tri_sb[:],
                            )
                    # row sums
                    ssum = psSum.tile([1, 512], F32)
                    for j in range(nk):
                        q0 = 128 * max(j - 4 * b, 0)
                        nc.tensor.matmul(
                            ssum[:, q0:512], ones_col[:], eS[:, j, q0:512],
                            start=(j == 0), stop=(j == nk - 1),
                        )
                    sums_bf = sm.tile([1, 512], BF16, tag="sums")
                    nc.scalar.copy(sums_bf[:], ssum[:])
                    recip = sm.tile([1, 512], F32, tag="recip")
                    nc.vector.reciprocal_approx_fast(recip[:], ssum[:])
                    bcast = sm.tile([128, 512], F32, tag="bcast")
                    nc.gpsimd.partition_broadcast(bcast[:], recip[:])
                    # PV (+ rank-1 bv*sums fold)
                    pso = psO.tile([128, 512], F32)
                    for j in range(nk):
                        q0 = 128 * max(j - 4 * b, 0)
                        nc.tensor.matmul(
                            pso[:, q0:512],
                            v_res[:, j, h * 128 : (h + 1) * 128],
                            eS[:, j, q0:512],
                            start=(j == 0), stop=False,
                        )
                    nc.tensor.matmul(
                        pso[:],
                        bv_sb[0:1, h * 128 : (h + 1) * 128],
                        sums_bf[:],
                        start=False, stop=True,
                    )
                    nc.vector.tensor_mul(ao_blk[:, h, :], pso[:], bcast[:])
                # out-projection for this block's 4 t-tiles
                for tl in range(4):
                    tt = 4 * b + tl
                    for eb in range(NT):
                        acc = ps3.tile([128, 512], F32)
                        for fo in range(NFO):
                            nc.tensor.matmul(
                                acc[:],
                                ao_blk[:, fo, tl * 128 : (tl + 1) * 128],
                                wo_sb[:, fo, eb * 512 : (eb + 1) * 512],
                                start=(fo == 0),
                                stop=(fo == NFO - 1),
                            )
                        osb = st.tile([128, 512], F32, tag="osb")
                        nc.vector.tensor_copy(osb[:], acc[:])
                        # round-robin output stores across the 3 DMA queues
                        # so the final drain isn't serialized on one ring
                        qeng = (nc.sync, nc.gpsimd, nc.scalar)[(4 * tt + eb) % 3]
                        qeng.dma_start(
                            out[tt * 128 : (tt + 1) * 128,
                                eb * 512 : (eb + 1) * 512],
                            osb[:],
                        )

    nc.compile()
    return nc


def _host_inputs(x, Wq, bq, Wk, bk, Wv, bv, Wo, bo):
    BF = ml_dtypes.bfloat16
    # Per-head feature permutation: evens then odds (RoPE half-swap layout).
    perm1 = np.concatenate([np.arange(0, D, 2), np.arange(1, D, 2)])
    i = np.arange(0, D, 2, dtype=np.float64)
    invf = ROPE_BASE ** (-i / D)                      # (64,)
    pos = np.arange(S, dtype=np.float64)
    ang = pos[None, :] * invf[:, None]                # (64, S)
    cosT = np.concatenate([np.cos(ang), np.cos(ang)], 0).astype(np.float32)
    # sin table signs match the half-SWAPPED operand (raws): row i<64 holds
    # -sin_i (multiplies o_i), row 64+i holds +sin_i (multiplies e_i).
    sinST = np.concatenate([-np.sin(ang), np.sin(ang)], 0).astype(np.float32)
    ki = np.arange(128)[:, None]
    qi = np.arange(128)[None, :]
    tri = (qi >= ki).astype(np.float32)

    def tile_x(xb):
        # [p, tb*NE*512 + eo*512 + tl] = x[tb*512+tl, eo*128+p]
        return np.ascontiguousarray(
            xb.reshape(NT, 512, NE, 128).transpose(3, 0, 2, 1)
        ).reshape(128, NT * NE * 512)

    def tile_w(w_l):
        # w_l: [FL(out), E(in)] -> [p, half*NE*512 + eo*512 + f] = w_l.T[eo*128+p, half*512+f]
        return np.ascontiguousarray(
            w_l.T.reshape(NE, 128, 2, 512).transpose(1, 2, 0, 3)
        ).reshape(128, 2 * NE * 512)

    in_maps = []
    x_tiled = {}
    for c in range(N_CORES):
        bi, g = c % B, c // B
        sl = slice(g * FL, (g + 1) * FL)
        fperm = (np.arange(FL).reshape(NFO, D)[:, perm1]).reshape(FL)
        wq_l = Wq[sl, :][fperm, :]
        wk_l = Wk[sl, :][fperm, :]
        bq_l = bq[sl][fperm]
        bk_l = bk[sl][fperm]
        if bi not in x_tiled:
            x_tiled[bi] = tile_x(x[bi]).astype(BF)
        in_maps.append({
            "xT": x_tiled[bi],
            "wq": tile_w(wq_l).astype(BF),
            "wk": tile_w(wk_l).astype(BF),
            "wv": tile_w(Wv[sl, :]).astype(BF),
            "wo": np.ascontiguousarray(Wo[:, sl].T).astype(BF),
            "bq": np.ascontiguousarray(bq_l.reshape(NFO, 128).T).astype(np.float32),
            "bk": np.ascontiguousarray(bk_l.reshape(NFO, 128).T).astype(np.float32),
            "bv": bv[sl].reshape(1, FL).astype(BF),
            "cosT": cosT.astype(BF),
            "sinST": sinST.astype(BF),
            "tri": tri.astype(BF),
            "ones_col": np.ones((128, 1), np.float32).astype(BF),
        })
    return in_maps


_NC_CACHE = {}


def _install_ntff_hook():
    """Recreate the missing antenv.axon_hooks module so trace=True works."""
    import sys, types, ctypes, contextlib

    if "antenv.axon_hooks" in sys.modules:
        return
    so_path = "/opt/axon/libaxon_pjrt.so"
    lib = ctypes.CDLL(so_path)
    lib.axon_start_nrt_profile.argtypes = [
        ctypes.POINTER(ctypes.c_int64), ctypes.c_size_t,
    ]
    lib.axon_start_nrt_profile.restype = ctypes.c_int64
    lib.axon_stop_nrt_profile.argtypes = [ctypes.c_char_p]
    lib.axon_stop_nrt_profile.restype = ctypes.c_int64

    @contextlib.contextmanager
    def _hook(output_dir, device_ids):
        import jax
        jax.devices()
        if device_ids:
            ids = (ctypes.c_int64 * len(device_ids))(*device_ids)
            rc = lib.axon_start_nrt_profile(ids, len(device_ids))
        else:
            rc = lib.axon_start_nrt_profile(None, 0)
        if rc != 0:
            raise RuntimeError(f"axon_start_nrt_profile rc={rc}")
        try:
            yield
        finally:
            n = lib.axon_stop_nrt_profile(str(output_dir).encode())
            print(f"ntff profile: {n} file(s) -> {output_dir}", flush=True)

    mod = types.ModuleType("antenv.axon_hooks")
    mod.get_axon_ntff_profile_hook = lambda: _hook
    mod.set_axon_ntff_profile_hook = lambda h: None
    sys.modules["antenv.axon_hooks"] = mod
    import concourse.bass_utils as bu
    bu.upload_artifacts = lambda tmpdir: f"file://{tmpdir}"


def run(x, Wq, bq, Wk, bk, Wv, bv, Wo, bo, trace=False):
    if trace:
        try:
            _install_ntff_hook()
        except Exception as e:
            print(f"ntff hook install failed ({e}); tracing may degrade")
    if "nc" not in _NC_CACHE:
        _NC_CACHE["nc"] = build_nc()
    nc = _NC_CACHE["nc"]
    in_maps = _host_inputs(x, Wq, bq, Wk, bk, Wv, bv, Wo, bo)
    res = run_bass_kernel_spmd(
        nc, in_maps, core_ids=list(range(N_CORES)), trace=trace
    )
    outs = [res.results[c]["out"] for c in range(N_CORES)]
    full = np.empty((B, S, E), np.float32)
    for bi in range(B):
        full[bi] = outs[bi] + outs[bi + B] + bo[None, :]
    return full, res


def kernel(**inputs):
    full, _ = run(**inputs)
    return full


# revision 35
# speedup vs baseline: 1.1078x; 1.0091x over previous
"""Bass/Trainium2 kernel for nn_Attention_13615046328582.

Causal multi-head attention with RoPE, B=4 S=2048 E=2048 H=16 D=128, fp32 io.
Sharding: 4-way batch DP x 2-way head TP across 8 NeuronCores.
Each core: 1 batch, 8 heads. Host sums the TP pair partials + bo.

All matmuls bf16 (same PE rate as fp32r; FWL hides weight loads, DMA/SBUF
halve). fp8/DoubleRow was evaluated and rejected: numpy simulation of e4m3
on any single GEMM stage gives 2.7-4.6e-2 max-err metric, over the 2e-2
gate. bf16-everything sims at 4.2e-3 and measures 4.5e-3 on HW.

Measured (NTFF, warm 2.4GHz clock): 726us/core NEFF wall, PE ~94% busy
(vs 950us for the fp32r baseline). PE streaming floor is ~625us; the rest
is per-MM issue latency, ~20us cold-start DMA (HBM-aggregate bound), and
~12us teardown fences.
Beware: the shared device occasionally drops to 2.0GHz (P0 power state),
inflating runs by ~20% - check inter-MM gap (216ns = warm 2.4GHz).

Per-core plan:
  P1  V/Q/K projections; x streamed per 512-t block on the vector DMA queue
      (weights on sync/scalar queues so the cold-start loads run in
      parallel). Q/K features host-permuted to [evens|odds] per head so
      RoPE's pair swap is a half-partition swap: SBUF->SBUF DMA on the
      gpsimd queue + 3 DVE ops. Q/K land in per-head resident tiles (so P2
      head 0 doesn't wait on the whole pass); V resident too. No DRAM
      spill.
  P2+P3 fused, blocks outer / heads inner. Per (block, head): fine-grained
      causal logits (diagonal-straddle tiles stream only their valid
      q-suffix; true diagonal masked by a bf16 triangle mul), exp on ACT
      batched in 2-bank pairs, row sums via ones-vector matmuls, 1/sums via
      DVE reciprocal_approx_fast, broadcast via gpsimd partition_broadcast
      (no PE involvement), normalize on DVE straight into the block's ao
      tile. After each block's heads, the out-projection for its 4 t-tiles
      runs from SBUF (wo prefetched during block 0); f32 out.
"""

import numpy as np
import ml_dtypes
from contextlib import ExitStack

import concourse.bass as bass
import concourse.tile as tile
from concourse import bacc, mybir
from concourse.bass_utils import run_bass_kernel_spmd

B, S, E, H = 4, 2048, 2048, 16
D = E // H            # 128 head dim
HL = 8                # heads per core
FL = HL * D           # 1024 local features
N_CORES = 8
ROPE_BASE = 10000.0
SCALE = float(D) ** -0.5
F32 = mybir.dt.float32
F32R = mybir.dt.float32r
BF16 = mybir.dt.bfloat16
Act = mybir.ActivationFunctionType

NE = E // 128         # 16 e-tiles
NT = S // 512         # 4 t-blocks of 512
NTT = S // 128        # 16 t-tiles of 128
NFO = FL // 128       # 8 f-tiles (= local heads)


def build_nc():
    nc = bacc.Bacc(
        "TRN2", target_bir_lowering=False, debug=False, num_devices=N_CORES
    )
    # x and projection weights arrive host-pre-tiled so every chunk DMA is
    # a fully contiguous [128 x 8KB] transfer:
    #   xT[p, tb*NE*512 + eo*512 + tl] = x[tb*512+tl, eo*128+p]
    #   w*[p, half*NE*512 + eo*512 + f] = W.T[eo*128+p, half*512+f]
    xT = nc.dram_tensor("xT", [128, NT * NE * 512], BF16, kind="ExternalInput").ap()
    wq = nc.dram_tensor("wq", [128, 2 * NE * 512], BF16, kind="ExternalInput").ap()
    wk = nc.dram_tensor("wk", [128, 2 * NE * 512], BF16, kind="ExternalInput").ap()
    wv = nc.dram_tensor("wv", [128, 2 * NE * 512], BF16, kind="ExternalInput").ap()
    wo = nc.dram_tensor("wo", [FL, E], BF16, kind="ExternalInput").ap()
    bq = nc.dram_tensor("bq", [128, NFO], F32, kind="ExternalInput").ap()
    bk = nc.dram_tensor("bk", [128, NFO], F32, kind="ExternalInput").ap()
    bv = nc.dram_tensor("bv", [128, NFO], F32, kind="ExternalInput").ap()
    cosT = nc.dram_tensor("cosT", [128, S], BF16, kind="ExternalInput").ap()
    sinST = nc.dram_tensor("sinST", [128, S], BF16, kind="ExternalInput").ap()
    tri = nc.dram_tensor("tri", [128, 128], BF16, kind="ExternalInput").ap()
    ones_col_d = nc.dram_tensor("ones_col", [128, 1], BF16, kind="ExternalInput").ap()
    out = nc.dram_tensor("out", [S, E], F32, kind="ExternalOutput").ap()

    with tile.TileContext(nc) as tc, ExitStack() as top:
        cpool = top.enter_context(tc.tile_pool(name="const", bufs=1))
        ones_col = cpool.tile([128, 1], BF16, tag="ones_col")
        nc.sync.dma_start(ones_col[:], ones_col_d[:])
        zb = cpool.tile([128, 1], F32, tag="zb")
        nc.gpsimd.memset(zb[:], 0.0)
        # Warm the gpsimd Q7 partition_broadcast library now (first use pays
        # a ~12us program load; do it while PE waits on the initial DMAs).
        warm_in = cpool.tile([1, 8], F32, tag="warm_in")
        nc.gpsimd.memset(warm_in[:], 1.0)
        warm_out = cpool.tile([128, 8], F32, tag="warm_out")
        nc.gpsimd.partition_broadcast(warm_out[:], warm_in[:])
        tri_sb = cpool.tile([128, 128], BF16, tag="tri")
        nc.sync.dma_start(tri_sb[:], tri[:])
        bq_sb = cpool.tile([128, NFO], F32, tag="bq")
        nc.sync.dma_start(bq_sb[:], bq[:])
        bk_sb = cpool.tile([128, NFO], F32, tag="bk")
        nc.sync.dma_start(bk_sb[:], bk[:])
        bv_sb = cpool.tile([128, NFO], F32, tag="bv")
        nc.sync.dma_start(bv_sb[:], bv[:])

        # Whole-kernel residents. Q/K per head so P2's head h only waits on
        # head h's writes (tile-granular deps); V one tile (fully written
        # long before first use).
        res = top.enter_context(tc.tile_pool(name="res", bufs=1))
        qT_res = [res.tile([128, S], BF16, tag=f"qT{h}", name=f"qT{h}")
                  for h in range(NFO)]
        kT_res = [res.tile([128, S], BF16, tag=f"kT{h}", name=f"kT{h}")
                  for h in range(NFO)]
        v_res = res.tile([128, NTT, FL], BF16, tag="v")



        # ---- Phase 1: V/Q/K projections (V first: it gates nothing until
        # P2, but its weights are the cold-start load).
        with ExitStack() as ph:
            c1 = ph.enter_context(tc.tile_pool(name="c1", bufs=1))
            wp = ph.enter_context(tc.tile_pool(name="wqk", bufs=6))
            xp = ph.enter_context(tc.tile_pool(name="xs", bufs=4))
            ps = ph.enter_context(tc.tile_pool(name="ps1", bufs=6, space="PSUM"))
            st = ph.enter_context(tc.tile_pool(name="st1", bufs=3))

            # x and w stream as [128, 8, 512] eo-half chunks (separate tiles
            # so the first matmuls start as soon as their chunk lands, not
            # after the whole 2MB half); each chunk is one contiguous
            # [128 x 8KB] DMA thanks to the host pre-tiling.
            def load_w_half(w_in, half, engs=None):
                ts = []
                for c in range(2):
                    t = wp.tile([128, 8, 512], BF16, tag="w", name=f"w{c}")
                    off = (half * NE + c * 8) * 512
                    eng = engs[c] if engs else nc.sync
                    eng.dma_start(t[:], w_in[:, off : off + 8 * 512])
                    ts.append(t)
                return ts

            def load_x(tb, engs=None):
                ts = []
                for c in range(2):
                    t = xp.tile([128, 8, 512], BF16, tag="x", name=f"x{c}")
                    off = (tb * NE + c * 8) * 512
                    # scalar queue: parallel with the weight loads on sync
                    eng = engs[c] if engs else nc.scalar
                    eng.dma_start(t[:], xT[:, off : off + 8 * 512])
                    ts.append(t)
                return ts

            # Cold start is HBM/ring bound: stripe the critical first 6MB
            # (x block 0 + both wv halves) across all three DMA queues so
            # each ring carries ~2MB; wv1 chunks queue last per ring.
            wv_half = [load_w_half(wv, 0, engs=(nc.sync, nc.scalar))]
            x_sb = load_x(0, engs=(nc.scalar, nc.gpsimd))
            wv_half.append(load_w_half(wv, 1, engs=(nc.gpsimd, nc.sync)))
            for tb in range(NT):
                if tb > 0:
                    x_sb = load_x(tb)
                # tb=0 runs all fo2=0 tiles first: the first ~14us of MMs
                # only need wv half 0, halving the cold-start DMA pressure.
                order = ([(ttl, 0) for ttl in range(4)]
                         + [(ttl, 1) for ttl in range(4)]) if tb == 0 else \
                        [(ttl, fo2) for ttl in range(4) for fo2 in range(2)]
                for ttl, fo2 in order:
                    tt = tb * 4 + ttl
                    acc = ps.tile([128, 512], F32)
                    for eo in range(NE):
                        nc.tensor.matmul(
                            acc[:],
                            x_sb[eo // 8][:, eo % 8,
                                          ttl * 128 : (ttl + 1) * 128],
                            wv_half[fo2][eo // 8][:, eo % 8, :],
                            start=(eo == 0),
                            stop=(eo == NE - 1),
                        )
                    nc.scalar.copy(
                        v_res[:, tt, fo2 * 512 : (fo2 + 1) * 512], acc[:]
                    )

            cos_sb = c1.tile([128, S], BF16, tag="cos")
            nc.sync.dma_start(cos_sb[:], cosT[:])
            sin_sb = c1.tile([128, S], BF16, tag="sin")
            nc.sync.dma_start(sin_sb[:], sinST[:])

            # Q and K passes: bias via ACT evac; RoPE on DVE with the half-
            # partition swap done by SBUF->SBUF DMA on the gpsimd queue.
            for w_in, b_sb, dst in ((wq, bq_sb, qT_res), (wk, bk_sb, kT_res)):
                w_half = [load_w_half(w_in, 0), load_w_half(w_in, 1)]
                for tb in range(NT):
                    x_sb = load_x(tb)
                    ts = slice(tb * 512, (tb + 1) * 512)
                    for fo in range(NFO):
                        w_sb = w_half[fo // 4]
                        fl = (fo % 4) * 128
                        acc = ps.tile([128, 512], F32)
                        for eo in range(NE):
                            nc.tensor.matmul(
                                acc[:],
                                w_sb[eo // 8][:, eo % 8, fl : fl + 128],
                                x_sb[eo // 8][:, eo % 8, :],
                                start=(eo == 0),
                                stop=(eo == NE - 1),
                            )
                        raw = st.tile([128, 512], BF16, tag="raw")
                        nc.scalar.activation(
                            raw[:], acc[:], Act.Identity,
                            bias=b_sb[:, fo : fo + 1],
                        )
                        raws = st.tile([128, 512], BF16, tag="raws")
                        nc.gpsimd.dma_start(raws[0:64, :], raw[64:128, :])
                        nc.gpsimd.dma_start(raws[64:128, :], raw[0:64, :])
                        u = st.tile([128, 512], BF16, tag="u")
                        nc.vector.tensor_mul(u[:], raw[:], cos_sb[:, ts])
                        w2 = st.tile([128, 512], BF16, tag="w2")
                        nc.vector.tensor_mul(w2[:], raws[:], sin_sb[:, ts])
                        nc.vector.tensor_add(dst[fo][:, ts], u[:], w2[:])

        # ---- Phase 2+3 fused: attention blocks + immediate out-projection.
        wp3 = top.enter_context(tc.tile_pool(name="wo", bufs=1))
        wo_sb = wp3.tile([128, NFO, E], BF16)
        wo_r = wo.rearrange("(fo p) e -> p fo e", p=128)
        for fo in range(NFO):
            nc.sync.dma_start(wo_sb[:, fo, :], wo_r[:, fo, :])

        with ExitStack() as ph:
            aop = ph.enter_context(tc.tile_pool(name="aob", bufs=2))
            ep = ph.enter_context(tc.tile_pool(name="expS", bufs=2))
            psS = ph.enter_context(tc.tile_pool(name="psS", bufs=3, space="PSUM"))
            psSum = ph.enter_context(
                tc.tile_pool(name="psSum", bufs=1, space="PSUM")
            )
            psO = ph.enter_context(tc.tile_pool(name="psO", bufs=2, space="PSUM"))
            ps3 = ph.enter_context(tc.tile_pool(name="ps3", bufs=2, space="PSUM"))
            sm = ph.enter_context(tc.tile_pool(name="sm2", bufs=2))
            st = ph.enter_context(tc.tile_pool(name="st2", bufs=3))

            for b in range(NT):
                nk = 4 * b + 4
                ao_blk = aop.tile([128, NFO, 512], BF16)
                for h in range(HL):
                    eS = ep.tile([128, NTT, 512], BF16)
                    # logits + exp
                    for j in range(nk):
                        m = j - 4 * b
                        q0 = 128 * max(m, 0)
                        ls = psS.tile([128, 512], F32)
                        nc.tensor.matmul(
                            ls[:, q0:512],
                            kT_res[h][:, j * 128 : (j + 1) * 128],
                            qT_res[h][:, b * 512 + q0 : (b + 1) * 512],
                            start=True, stop=True,
                        )
                        nc.scalar.activation(
                            eS[:, j, q0:512], ls[:, q0:512], Act.Exp,
                            bias=zb[:, 0:1], scale=SCALE,
                        )
                        if m >= 0:  # true-diagonal 128x128 triangle
                            nc.vector.tensor_mul(
                                eS[:, j, q0 : q0 + 128],
                                eS[:, j, q0 : q0 + 128],
                                tri_sb[:],
                            )
                    # row sums
                    ssum = psSum.tile([1, 512], F32)
                    for j in range(nk):
                        q0 = 128 * max(j - 4 * b, 0)
                        nc.tensor.matmul(
                            ssum[:, q0:512], ones_col[:], eS[:, j, q0:512],
                            start=(j == 0), stop=(j == nk - 1),
                        )
                    recip = sm.tile([1, 512], F32, tag="recip")
                    nc.vector.reciprocal_approx_fast(recip[:], ssum[:])
                    bcast = sm.tile([128, 512], F32, tag="bcast")
                    nc.gpsimd.partition_broadcast(bcast[:], recip[:])
                    # PV
                    pso = psO.tile([128, 512], F32)
                    for j in range(nk):
                        q0 = 128 * max(j - 4 * b, 0)
                        nc.tensor.matmul(
                            pso[:, q0:512],
                            v_res[:, j, h * 128 : (h + 1) * 128],
                            eS[:, j, q0:512],
                            start=(j == 0), stop=(j == nk - 1),
                        )
                    # normalize on DVE, then v-bias as a per-partition ACT
                    # bias (ao = pso/sums + bv) - no PE involvement
                    nrm = st.tile([128, 512], BF16, tag="nrm")
                    nc.vector.tensor_mul(nrm[:], pso[:], bcast[:])
                    nc.scalar.activation(
                        ao_blk[:, h, :], nrm[:], Act.Identity,
                        bias=bv_sb[:, h : h + 1],
                    )
                # out-projection for this block's 4 t-tiles
                for tl in range(4):
                    tt = 4 * b + tl
                    for eb in range(NT):
                        acc = ps3.tile([128, 512], F32)
                        for fo in range(NFO):
                            nc.tensor.matmul(
                                acc[:],
                                ao_blk[:, fo, tl * 128 : (tl + 1) * 128],
                                wo_sb[:, fo, eb * 512 : (eb + 1) * 512],
                                start=(fo == 0),
                                stop=(fo == NFO - 1),
                            )
                        osb = st.tile([128, 512], F32, tag="osb")
                        nc.vector.tensor_copy(osb[:], acc[:])
                        # round-robin output stores across the 3 DMA queues
                        # so the final drain isn't serialized on one ring
                        qeng = (nc.sync, nc.gpsimd, nc.scalar)[(4 * tt + eb) % 3]
                        qeng.dma_start(
                            out[tt * 128 : (tt + 1) * 128,
                                eb * 512 : (eb + 1) * 512],
                            osb[:],
                        )

    nc.compile()
    return nc


def _host_inputs(x, Wq, bq, Wk, bk, Wv, bv, Wo, bo):
    BF = ml_dtypes.bfloat16
    # Per-head feature permutation: evens then odds (RoPE half-swap layout).
    perm1 = np.concatenate([np.arange(0, D, 2), np.arange(1, D, 2)])
    i = np.arange(0, D, 2, dtype=np.float64)
    invf = ROPE_BASE ** (-i / D)                      # (64,)
    pos = np.arange(S, dtype=np.float64)
    ang = pos[None, :] * invf[:, None]                # (64, S)
    cosT = np.concatenate([np.cos(ang), np.cos(ang)], 0).astype(np.float32)
    # sin table signs match the half-SWAPPED operand (raws): row i<64 holds
    # -sin_i (multiplies o_i), row 64+i holds +sin_i (multiplies e_i).
    sinST = np.concatenate([-np.sin(ang), np.sin(ang)], 0).astype(np.float32)
    ki = np.arange(128)[:, None]
    qi = np.arange(128)[None, :]
    tri = (qi >= ki).astype(np.float32)

    def tile_x(xb):
        # [p, tb*NE*512 + eo*512 + tl] = x[tb*512+tl, eo*128+p]
        return np.ascontiguousarray(
            xb.reshape(NT, 512, NE, 128).transpose(3, 0, 2, 1)
        ).reshape(128, NT * NE * 512)

    def tile_w(w_l):
        # w_l: [FL(out), E(in)] -> [p, half*NE*512 + eo*512 + f] = w_l.T[eo*128+p, half*512+f]
        return np.ascontiguousarray(
            w_l.T.reshape(NE, 128, 2, 512).transpose(1, 2, 0, 3)
        ).reshape(128, 2 * NE * 512)

    in_maps = []
    x_tiled = {}
    for c in range(N_CORES):
        bi, g = c % B, c // B
        sl = slice(g * FL, (g + 1) * FL)
        fperm = (np.arange(FL).reshape(NFO, D)[:, perm1]).reshape(FL)
        wq_l = Wq[sl, :][fperm, :]
        wk_l = Wk[sl, :][fperm, :]
        bq_l = bq[sl][fperm]
        bk_l = bk[sl][fperm]
        if bi not in x_tiled:
            x_tiled[bi] = tile_x(x[bi]).astype(BF)
        in_maps.append({
            "xT": x_tiled[bi],
            "wq": tile_w(wq_l).astype(BF),
            "wk": tile_w(wk_l).astype(BF),
            "wv": tile_w(Wv[sl, :]).astype(BF),
            "wo": np.ascontiguousarray(Wo[:, sl].T).astype(BF),
            "bq": np.ascontiguousarray(bq_l.reshape(NFO, 128).T).astype(np.float32),
            "bk": np.ascontiguousarray(bk_l.reshape(NFO, 128).T).astype(np.float32),
            "bv": np.ascontiguousarray(bv[sl].reshape(NFO, 128).T).astype(np.float32),
            "cosT": cosT.astype(BF),
            "sinST": sinST.astype(BF),
            "tri": tri.astype(BF),
            "ones_col": np.ones((128, 1), np.float32).astype(BF),
        })
    return in_maps


_NC_CACHE = {}


def _install_ntff_hook():
    """Recreate the missing antenv.axon_hooks module so trace=True works."""
    import sys, types, ctypes, contextlib

    if "antenv.axon_hooks" in sys.modules:
        return
    so_path = "/opt/axon/libaxon_pjrt.so"
    lib = ctypes.CDLL(so_path)
    lib.axon_start_nrt_profile.argtypes = [
        ctypes.POINTER(ctypes.c_int64), ctypes.c_size_t,
    ]
    lib.axon_start_nrt_profile.restype = ctypes.c_int64
    lib.axon_stop_nrt_profile.argtypes = [ctypes.c_char_p]
    lib.axon_stop_nrt_profile.restype = ctypes.c_int64

    @contextlib.contextmanager
    def _hook(output_dir, device_ids):
        import jax
        jax.devices()
        if device_ids:
            ids = (ctypes.c_int64 * len(device_ids))(*device_ids)
            rc = lib.axon_start_nrt_profile(ids, len(device_ids))
        else:
            rc = lib.axon_start_nrt_profile(None, 0)
        if rc != 0:
            raise RuntimeError(f"axon_start_nrt_profile rc={rc}")
        try:
            yield
        finally:
            n = lib.axon_stop_nrt_profile(str(output_dir).encode())
            print(f"ntff profile: {n} file(s) -> {output_dir}", flush=True)

    mod = types.ModuleType("antenv.axon_hooks")
    mod.get_axon_ntff_profile_hook = lambda: _hook
    mod.set_axon_ntff_profile_hook = lambda h: None
    sys.modules["antenv.axon_hooks"] = mod
    import concourse.bass_utils as bu
    bu.upload_artifacts = lambda tmpdir: f"file://{tmpdir}"


def run(x, Wq, bq, Wk, bk, Wv, bv, Wo, bo, trace=False):
    if trace:
        try:
            _install_ntff_hook()
        except Exception as e:
            print(f"ntff hook install failed ({e}); tracing may degrade")
    if "nc" not in _NC_CACHE:
        _NC_CACHE["nc"] = build_nc()
    nc = _NC_CACHE["nc"]
    in_maps = _host_inputs(x, Wq, bq, Wk, bk, Wv, bv, Wo, bo)
    res = run_bass_kernel_spmd(
        nc, in_maps, core_ids=list(range(N_CORES)), trace=trace
    )
    outs = [res.results[c]["out"] for c in range(N_CORES)]
    full = np.empty((B, S, E), np.float32)
    for bi in range(B):
        full[bi] = outs[bi] + outs[bi + B] + bo[None, :]
    return full, res


def kernel(**inputs):
    full, _ = run(**inputs)
    return full
